# revision 23
# baseline (speedup 1.0000x reference)
"""Sparse (class-gated bilinear) attention kernel for TRN2, 8 NeuronCores.

Problem shapes (hardcoded): b=2, h=8, s=512, d=64, C=8 classes, B=4 bases.

Math (per b,h):
  W1e[c] = (sum_B softmax(alpha1)[c,B,h] * W1[B,h]) / sqrt(d)   (host)
  W2e[c] = sum_B softmax(alpha2)[c,B,h] * W2[B,h]               (host)
  UT_c   = W1e[c]^T-contraction:  UT_c[n,i] = sum_m W1e[c][m,n] * Q[i,m]
  ST_c   = ST_c[j,i] = sum_n K[j,n] * UT_c[n,i]                 (PE, fp32r)
  scoresT[j,i] = ST_{bmat[i,j]}[j,i] + rpb[i,j]                 (DVE select)
  E = exp(scoresT)           (no max-subtraction needed; |scores| < ~40)
  t_c[j,D] = sum_d V[j,d] W2e[c][d,D]                           (PE)
  outT[D,i] = sum_c sum_j t_c[j,D] * (E . mask_c)[j,i]          (PE, bf16)
  Z[i] = sum_j E[j,i]                                           (PE ones-row)
  out[i,D] = outT[D,i] / Z[i]                                   (host)

Sharding: 16 (b,h) pairs over 8 cores; core k handles b=k//4,
heads (2*(k%4), 2*(k%4)+1). b_mat shared by both heads of a core.
"""

import os
import sys

import numpy as np

if "/opt/trn_rl_repo" not in sys.path:
    sys.path.insert(0, "/opt/trn_rl_repo")

import ml_dtypes

B_, H_, S_, D_, C_ = 2, 8, 512, 64, 8
NCORES = 8
JT = S_ // 128  # 4 j-tiles

# Selection chain dtype is fp32 (reads ST PSUM directly); split/output
# matmul side runs in ELEM (bf16 — contributes only ~0.2% rel err).
ELEM = "bfloat16"

_CACHE = {}


def _softmax(a, axis):
    e = np.exp(a - a.max(axis=axis, keepdims=True))
    return e / e.sum(axis=axis, keepdims=True)


def _build_nc():
    import concourse.bass as bass  # noqa: F401
    import concourse.mybir as mybir
    from concourse import bacc
    from concourse.tile import TileContext

    f32 = mybir.dt.float32
    f32r = mybir.dt.float32r
    f16 = mybir.dt.float16
    ebt = mybir.dt.bfloat16 if ELEM == "bfloat16" else mybir.dt.float32

    nc = bacc.Bacc("TRN2", target_bir_lowering=False, debug=False)

    qt_d = nc.dram_tensor("qt", [2, 64, 512], f32r, kind="ExternalInput").ap()
    kt_d = nc.dram_tensor("kt", [2, 64, 512], f32r, kind="ExternalInput").ap()
    vt_d = nc.dram_tensor("vt", [2, 64, 512], f32r, kind="ExternalInput").ap()
    w1_d = nc.dram_tensor("w1", [2, 64, 512], f32r, kind="ExternalInput").ap()
    w2_d = nc.dram_tensor("w2", [2, 64, 512], f32r, kind="ExternalInput").ap()
    erp_d = nc.dram_tensor("erp", [2, 512, 512], ebt, kind="ExternalInput").ap()
    bmt_d = nc.dram_tensor("bmt", [512, 512], ebt, kind="ExternalInput").ap()
    ot_d = nc.dram_tensor("ot", [2, 64, 512], f32, kind="ExternalOutput").ap()
    z_d = nc.dram_tensor("z", [2, 1, 512], f32, kind="ExternalOutput").ap()

    EXP = mybir.ActivationFunctionType.Exp
    EQ = mybir.AluOpType.is_equal

    with TileContext(nc) as tc:
        with (
            tc.tile_pool(name="const", bufs=1) as cpool,
            tc.tile_pool(name="inp", bufs=1) as ipool,
            tc.tile_pool(name="mask", bufs=1) as mpool,
            tc.tile_pool(name="work", bufs=4) as wpool,
            tc.tile_pool(name="ec", bufs=20) as epool,
            tc.tile_pool(name="pst", bufs=4, space="PSUM") as pst,
            tc.tile_pool(name="pacc", bufs=1, space="PSUM") as pacc,
        ):
            ones = cpool.tile([128, 1], ebt, tag="ones")
            nc.vector.memset(ones, 1.0)

            # per-class uint16 masks from b_mat^T, shared by both heads on
            # the core; used by copy_predicated (chain) AND split muls
            imasks = [[None] * C_ for _ in range(JT)]
            for jt in range(JT):
                bt = ipool.tile([128, 512], ebt, tag=f"bmt{jt}")
                nc.sync.dma_start(out=bt, in_=bmt_d[jt * 128 : (jt + 1) * 128, :])
                for c in range(C_):
                    im = mpool.tile([128, 512], mybir.dt.uint16, tag=f"i{jt}_{c}")
                    nc.vector.tensor_scalar(im, bt, float(c), None, EQ)
                    imasks[jt][c] = im

            qt, kt, vt, w1, w2 = {}, {}, {}, {}, {}
            ut, tsb = {}, {}
            ot_ps, z_ps = {}, {}
            for p in range(2):
                qt[p] = ipool.tile([64, 512], f32r, tag=f"qt{p}", name=f"qt{p}")
                nc.sync.dma_start(out=qt[p], in_=qt_d[p])
                kt[p] = ipool.tile([64, 512], f32r, tag=f"kt{p}", name=f"kt{p}")
                nc.sync.dma_start(out=kt[p], in_=kt_d[p])
                vt[p] = ipool.tile([64, 512], f32r, tag=f"vt{p}", name=f"vt{p}")
                nc.sync.dma_start(out=vt[p], in_=vt_d[p])
                w1[p] = ipool.tile([64, 512], f32r, tag=f"w1{p}", name=f"w1{p}")
                nc.sync.dma_start(out=w1[p], in_=w1_d[p])
                w2[p] = ipool.tile([64, 512], f32r, tag=f"w2{p}", name=f"w2{p}")
                nc.sync.dma_start(out=w2[p], in_=w2_d[p])

                # UT_c = W1e[c].T-contract @ Q^T : [64, 512] each
                ut[p] = []
                for c in range(C_):
                    up = pst.tile([128, 512], mybir.dt.float32, tag="st")
                    nc.tensor.matmul(
                        up[:64], w1[p][:, c * 64 : (c + 1) * 64], qt[p],
                        start=True, stop=True,
                    )
                    us = ipool.tile([64, 512], f32r, tag=f"ut{p}_{c}")
                    nc.any.tensor_copy(out=us, in_=up[:64])
                    ut[p].append(us)

                # t_all[j-tile] = V-tile @ W2cat : [128, (c,D)=512]
                tsb[p] = []
                for jt in range(JT):
                    tp = pst.tile([128, 512], mybir.dt.float32, tag="st")
                    nc.tensor.matmul(
                        tp, vt[p][:, jt * 128 : (jt + 1) * 128], w2[p],
                        start=True, stop=True,
                    )
                    ts = ipool.tile([128, 512], ebt, tag=f"t{p}_{jt}")
                    nc.any.tensor_copy(out=ts, in_=tp)
                    tsb[p].append(ts)

                ot_ps[p] = pacc.tile([64, 512], mybir.dt.float32, tag=f"o{p}", name=f"ot{p}")
                z_ps[p] = pacc.tile([1, 512], mybir.dt.float32, tag=f"z{p}", name=f"zp{p}")

            # Interleaved (jt, p) steps. Output/Z matmuls for step s are
            # emitted during step s+1 so they never block the next step's
            # ST matmuls in the in-order PE stream.
            pending = None

            def flush_pending():
                et_, ecs_, p_, jt_ = pending
                nc.tensor.matmul(
                    z_ps[p_], ones, et_,
                    start=(jt_ == 0), stop=(jt_ == JT - 1),
                    skip_group_check=True,
                )
                for c in range(C_):
                    nc.tensor.matmul(
                        ot_ps[p_], tsb[p_][jt_][:, c * 64 : (c + 1) * 64],
                        ecs_[c],
                        start=(jt_ == 0 and c == 0),
                        stop=(jt_ == JT - 1 and c == C_ - 1),
                        skip_group_check=True,
                    )

            for jt in range(JT):
                for p in range(2):
                    rp = wpool.tile([128, 512], ebt, tag="rpb")
                    nc.sync.dma_start(
                        out=rp, in_=erp_d[p, jt * 128 : (jt + 1) * 128, :]
                    )
                    # ST matmuls -> PSUM; fp32 selection chain reads the
                    # PSUM banks directly. ACT does the class-0 seed copy.
                    sc = wpool.tile([128, 512], f32, tag="sc")
                    for c in range(C_):
                        sp = pst.tile([128, 512], mybir.dt.float32, tag="st")
                        nc.tensor.matmul(
                            sp, kt[p][:, jt * 128 : (jt + 1) * 128], ut[p][c],
                            start=True, stop=True,
                        )
                        if c == 0:
                            nc.scalar.copy(sc, sp)
                        else:
                            nc.vector.copy_predicated(sc, imasks[jt][c], sp)

                    eraw = wpool.tile([128, 512], ebt, tag="eraw")
                    nc.scalar.activation(eraw, sc, EXP)
                    et = wpool.tile([128, 512], ebt, tag="et")
                    nc.vector.tensor_mul(et, eraw, rp)

                    ecs = []
                    for c in range(C_):
                        ec = epool.tile([128, 512], ebt, tag="ec")
                        eng = nc.gpsimd if c in (2, 4, 6) else nc.vector
                        eng.tensor_mul(ec, et, imasks[jt][c])
                        ecs.append(ec)

                    if pending is not None:
                        flush_pending()
                    pending = (et, ecs, p, jt)
            flush_pending()

            for p in range(2):
                os_ = wpool.tile([64, 512], mybir.dt.float32, tag="os")
                nc.scalar.copy(os_, ot_ps[p])
                nc.sync.dma_start(out=ot_d[p], in_=os_)
                zs = wpool.tile([1, 512], mybir.dt.float32, tag="zs")
                nc.scalar.copy(zs, z_ps[p])
                nc.sync.dma_start(out=z_d[p], in_=zs)

    nc.compile()
    return nc


def _get_nc():
    if "nc" not in _CACHE:
        _CACHE["nc"] = _build_nc()
    return _CACHE["nc"]


def kernel(**inputs):
    q = np.asarray(inputs["query"], np.float32)
    k = np.asarray(inputs["key"], np.float32)
    v = np.asarray(inputs["value"], np.float32)
    bm = np.asarray(inputs["b_mat"])
    rpb = np.asarray(inputs["rpb"], np.float32)
    W1 = np.asarray(inputs["W1"], np.float32)
    a1 = np.asarray(inputs["alpha1"], np.float32)
    W2 = np.asarray(inputs["W2"], np.float32)
    a2 = np.asarray(inputs["alpha2"], np.float32)
    mask = np.asarray(inputs["mask"])

    W1e = np.einsum("Bhmn,CBh->Chmn", W1, _softmax(a1, 1)) / np.sqrt(D_)
    W2e = np.einsum("BhdD,CBh->ChdD", W2, _softmax(a2, 1))

    eb = ml_dtypes.bfloat16 if ELEM == "bfloat16" else np.float32
    # additive -inf pair mask would go here; spec guarantees mask == ones
    assert mask.all(), "kernel assumes all-ones mask (spec fill=ones)"

    in_maps = []
    for cid in range(NCORES):
        b = cid // 4
        hs = [2 * (cid % 4), 2 * (cid % 4) + 1]
        qt = np.stack([q[b, h].T for h in hs]).astype(np.float32)
        kt = np.stack([k[b, h].T for h in hs]).astype(np.float32)
        vt = np.stack([v[b, h].T for h in hs]).astype(np.float32)
        # [m, C, n] -> [64, 512] per head
        w1 = np.stack(
            [W1e[:, h].transpose(1, 0, 2).reshape(64, 512) for h in hs]
        ).astype(np.float32)
        w2 = np.stack(
            [W2e[:, h].transpose(1, 0, 2).reshape(64, 512) for h in hs]
        ).astype(np.float32)
        erp = np.exp(np.stack([rpb[b, h].T for h in hs])).astype(
            ml_dtypes.bfloat16
        )
        bmt = bm[b].T.astype(np.float32).astype(eb)
        in_maps.append(
            {"qt": qt, "kt": kt, "vt": vt, "w1": w1, "w2": w2,
             "erp": erp, "bmt": bmt}
        )

    import time

    from concourse.bass_utils import run_bass_kernel_spmd

    try:
        res = run_bass_kernel_spmd(
            _get_nc(), in_maps, core_ids=list(range(NCORES))
        )
    except Exception:
        # transient NRT_EXEC_UNIT_UNRECOVERABLE from a previously wedged
        # device clears on redispatch
        time.sleep(5)
        res = run_bass_kernel_spmd(
            _get_nc(), in_maps, core_ids=list(range(NCORES))
        )
    _CACHE["last_res"] = res
    outs = res.results

    out = np.zeros((B_, H_, S_, D_), np.float32)
    for cid in range(NCORES):
        b = cid // 4
        hs = [2 * (cid % 4), 2 * (cid % 4) + 1]
        for p, h in enumerate(hs):
            ot = np.asarray(outs[cid]["ot"][p], np.float32)  # [64, 512]
            z = np.asarray(outs[cid]["z"][p], np.float32)  # [1, 512]
            out[b, h] = (ot / z).T
    return out


# revision 25
# speedup vs baseline: 1.0630x; 1.0630x over previous
"""Sparse (class-gated bilinear) attention kernel for TRN2, 8 NeuronCores.

Problem shapes (hardcoded): b=2, h=8, s=512, d=64, C=8 classes, B=4 bases.

Math (per b,h):
  W1e[c] = (sum_B softmax(alpha1)[c,B,h] * W1[B,h]) / sqrt(d)   (host)
  W2e[c] = sum_B softmax(alpha2)[c,B,h] * W2[B,h]               (host)
  UT_c   = W1e[c]^T-contraction:  UT_c[n,i] = sum_m W1e[c][m,n] * Q[i,m]
  ST_c   = ST_c[j,i] = sum_n K[j,n] * UT_c[n,i]                 (PE, fp32r)
  scoresT[j,i] = ST_{bmat[i,j]}[j,i] + rpb[i,j]                 (DVE select)
  E = exp(scoresT)           (no max-subtraction needed; |scores| < ~40)
  t_c[j,D] = sum_d V[j,d] W2e[c][d,D]                           (PE)
  outT[D,i] = sum_c sum_j t_c[j,D] * (E . mask_c)[j,i]          (PE, bf16)
  Z[i] = sum_j E[j,i]                                           (PE ones-row)
  out[i,D] = outT[D,i] / Z[i]                                   (host)

Sharding: 16 (b,h) pairs over 8 cores; core k handles b=k//4,
heads (2*(k%4), 2*(k%4)+1). b_mat shared by both heads of a core.
"""

import os
import sys

import numpy as np

if "/opt/trn_rl_repo" not in sys.path:
    sys.path.insert(0, "/opt/trn_rl_repo")

import ml_dtypes

B_, H_, S_, D_, C_ = 2, 8, 512, 64, 8
NCORES = 8
JT = S_ // 128  # 4 j-tiles

# Selection chain dtype is fp32 (reads ST PSUM directly); split/output
# matmul side runs in ELEM (bf16 — contributes only ~0.2% rel err).
ELEM = "bfloat16"

_CACHE = {}


def _softmax(a, axis):
    e = np.exp(a - a.max(axis=axis, keepdims=True))
    return e / e.sum(axis=axis, keepdims=True)


def _build_nc():
    import concourse.bass as bass  # noqa: F401
    import concourse.mybir as mybir
    from concourse import bacc
    from concourse.tile import TileContext

    f32 = mybir.dt.float32
    f32r = mybir.dt.float32r
    f16 = mybir.dt.float16
    ebt = mybir.dt.bfloat16 if ELEM == "bfloat16" else mybir.dt.float32

    nc = bacc.Bacc("TRN2", target_bir_lowering=False, debug=False)

    qt_d = nc.dram_tensor("qt", [2, 64, 512], f32r, kind="ExternalInput").ap()
    kt_d = nc.dram_tensor("kt", [2, 64, 512], f32r, kind="ExternalInput").ap()
    vt_d = nc.dram_tensor("vt", [2, 64, 512], f32r, kind="ExternalInput").ap()
    w1_d = nc.dram_tensor("w1", [2, 64, 512], f32r, kind="ExternalInput").ap()
    w2_d = nc.dram_tensor("w2", [2, 64, 512], f32r, kind="ExternalInput").ap()
    erp_d = nc.dram_tensor("erp", [2, 512, 512], ebt, kind="ExternalInput").ap()
    bmt_d = nc.dram_tensor("bmt", [512, 512], ebt, kind="ExternalInput").ap()
    ot_d = nc.dram_tensor("ot", [2, 65, 512], f32, kind="ExternalOutput").ap()

    EXP = mybir.ActivationFunctionType.Exp
    EQ = mybir.AluOpType.is_equal

    with TileContext(nc) as tc:
        with (
            tc.tile_pool(name="const", bufs=1) as cpool,
            tc.tile_pool(name="inp", bufs=1) as ipool,
            tc.tile_pool(name="mask", bufs=1) as mpool,
            tc.tile_pool(name="work", bufs=5) as wpool,
            tc.tile_pool(name="ec", bufs=24) as epool,
            tc.tile_pool(name="pst", bufs=6, space="PSUM") as pst,
            tc.tile_pool(name="pacc", bufs=1, space="PSUM") as pacc,
        ):
            ones = cpool.tile([128, 1], ebt, tag="ones")
            nc.vector.memset(ones, 1.0)

            # per-class uint16 masks from b_mat^T, shared by both heads on
            # the core; used by copy_predicated (chain) AND split muls
            imasks = [[None] * C_ for _ in range(JT)]
            for jt in range(JT):
                bt = ipool.tile([128, 512], ebt, tag=f"bmt{jt}")
                nc.sync.dma_start(out=bt, in_=bmt_d[jt * 128 : (jt + 1) * 128, :])
                for c in range(C_):
                    im = mpool.tile([128, 512], mybir.dt.uint16, tag=f"i{jt}_{c}")
                    nc.vector.tensor_scalar(im, bt, float(c), None, EQ)
                    imasks[jt][c] = im

            qt, kt, vt, w1, w2 = {}, {}, {}, {}, {}
            ut, tsb = {}, {}
            ot_ps, z_ps = {}, {}
            for p in range(2):
                qt[p] = ipool.tile([64, 512], f32r, tag=f"qt{p}", name=f"qt{p}")
                nc.sync.dma_start(out=qt[p], in_=qt_d[p])
                kt[p] = ipool.tile([64, 512], f32r, tag=f"kt{p}", name=f"kt{p}")
                nc.sync.dma_start(out=kt[p], in_=kt_d[p])
                vt[p] = ipool.tile([64, 512], f32r, tag=f"vt{p}", name=f"vt{p}")
                nc.sync.dma_start(out=vt[p], in_=vt_d[p])
                w1[p] = ipool.tile([64, 512], f32r, tag=f"w1{p}", name=f"w1{p}")
                nc.sync.dma_start(out=w1[p], in_=w1_d[p])
                w2[p] = ipool.tile([64, 512], f32r, tag=f"w2{p}", name=f"w2{p}")
                nc.sync.dma_start(out=w2[p], in_=w2_d[p])

                # UT_c = W1e[c].T-contract @ Q^T : [64, 512] each
                ut[p] = []
                for c in range(C_):
                    up = pst.tile([128, 512], mybir.dt.float32, tag="st")
                    nc.tensor.matmul(
                        up[:64], w1[p][:, c * 64 : (c + 1) * 64], qt[p],
                        start=True, stop=True,
                    )
                    us = ipool.tile([64, 512], f32r, tag=f"ut{p}_{c}")
                    nc.any.tensor_copy(out=us, in_=up[:64])
                    ut[p].append(us)

                # t_all[j-tile] = V-tile @ W2cat : [128, (c,D)=512]
                tsb[p] = []
                for jt in range(JT):
                    tp = pst.tile([128, 512], mybir.dt.float32, tag="st")
                    nc.tensor.matmul(
                        tp, vt[p][:, jt * 128 : (jt + 1) * 128], w2[p],
                        start=True, stop=True,
                    )
                    ts = ipool.tile([128, 512], ebt, tag=f"t{p}_{jt}")
                    nc.any.tensor_copy(out=ts, in_=tp)
                    tsb[p].append(ts)

                ot_ps[p] = pacc.tile([65, 512], mybir.dt.float32, tag=f"o{p}", name=f"ot{p}")
                z_ps[p] = ot_ps[p][64:65]

            # Interleaved (jt, p) steps. Output/Z matmuls for step s are
            # emitted during step s+1 so they never block the next step's
            # ST matmuls in the in-order PE stream.
            pending = None

            def flush_pending():
                et_, ecs_, p_, jt_ = pending
                nc.tensor.matmul(
                    z_ps[p_], ones, et_,
                    start=(jt_ == 0), stop=(jt_ == JT - 1),
                    skip_group_check=True,
                )
                for c in range(C_):
                    nc.tensor.matmul(
                        ot_ps[p_][:64], tsb[p_][jt_][:, c * 64 : (c + 1) * 64],
                        ecs_[c],
                        start=(jt_ == 0 and c == 0),
                        stop=(jt_ == JT - 1 and c == C_ - 1),
                        skip_group_check=True,
                    )

            for jt in range(JT):
                for p in range(2):
                    rp = wpool.tile([128, 512], ebt, tag="rpb")
                    nc.sync.dma_start(
                        out=rp, in_=erp_d[p, jt * 128 : (jt + 1) * 128, :]
                    )
                    # ST matmuls -> PSUM; fp32 selection chain reads the
                    # PSUM banks directly. ACT does the class-0 seed copy.
                    sc = wpool.tile([128, 512], f32, tag="sc")
                    for c in range(C_):
                        sp = pst.tile([128, 512], mybir.dt.float32, tag="st")
                        nc.tensor.matmul(
                            sp, kt[p][:, jt * 128 : (jt + 1) * 128], ut[p][c],
                            start=True, stop=True,
                        )
                        if c == 0:
                            nc.scalar.copy(sc, sp)
                        else:
                            nc.vector.copy_predicated(sc, imasks[jt][c], sp)

                    eraw = wpool.tile([128, 512], ebt, tag="eraw")
                    nc.scalar.activation(eraw, sc, EXP)
                    et = wpool.tile([128, 512], ebt, tag="et")
                    nc.vector.tensor_mul(et, eraw, rp)

                    ecs = []
                    for c in range(C_):
                        ec = epool.tile([128, 512], ebt, tag="ec")
                        eng = nc.gpsimd if c in (2, 4, 6) else nc.vector
                        eng.tensor_mul(ec, et, imasks[jt][c])
                        ecs.append(ec)

                    if pending is not None:
                        flush_pending()
                    pending = (et, ecs, p, jt)
            flush_pending()

            for p in range(2):
                os_ = wpool.tile([65, 512], mybir.dt.float32, tag="os")
                nc.scalar.copy(os_, ot_ps[p])
                nc.sync.dma_start(out=ot_d[p], in_=os_)

    nc.compile()
    return nc


def _get_nc():
    if "nc" not in _CACHE:
        _CACHE["nc"] = _build_nc()
    return _CACHE["nc"]


def kernel(**inputs):
    q = np.asarray(inputs["query"], np.float32)
    k = np.asarray(inputs["key"], np.float32)
    v = np.asarray(inputs["value"], np.float32)
    bm = np.asarray(inputs["b_mat"])
    rpb = np.asarray(inputs["rpb"], np.float32)
    W1 = np.asarray(inputs["W1"], np.float32)
    a1 = np.asarray(inputs["alpha1"], np.float32)
    W2 = np.asarray(inputs["W2"], np.float32)
    a2 = np.asarray(inputs["alpha2"], np.float32)
    mask = np.asarray(inputs["mask"])

    W1e = np.einsum("Bhmn,CBh->Chmn", W1, _softmax(a1, 1)) / np.sqrt(D_)
    W2e = np.einsum("BhdD,CBh->ChdD", W2, _softmax(a2, 1))

    eb = ml_dtypes.bfloat16 if ELEM == "bfloat16" else np.float32
    # additive -inf pair mask would go here; spec guarantees mask == ones
    assert mask.all(), "kernel assumes all-ones mask (spec fill=ones)"

    in_maps = []
    for cid in range(NCORES):
        b = cid // 4
        hs = [2 * (cid % 4), 2 * (cid % 4) + 1]
        qt = np.stack([q[b, h].T for h in hs]).astype(np.float32)
        kt = np.stack([k[b, h].T for h in hs]).astype(np.float32)
        vt = np.stack([v[b, h].T for h in hs]).astype(np.float32)
        # [m, C, n] -> [64, 512] per head
        w1 = np.stack(
            [W1e[:, h].transpose(1, 0, 2).reshape(64, 512) for h in hs]
        ).astype(np.float32)
        w2 = np.stack(
            [W2e[:, h].transpose(1, 0, 2).reshape(64, 512) for h in hs]
        ).astype(np.float32)
        erp = np.exp(np.stack([rpb[b, h].T for h in hs])).astype(
            ml_dtypes.bfloat16
        )
        bmt = bm[b].T.astype(np.float32).astype(eb)
        in_maps.append(
            {"qt": qt, "kt": kt, "vt": vt, "w1": w1, "w2": w2,
             "erp": erp, "bmt": bmt}
        )

    import time

    from concourse.bass_utils import run_bass_kernel_spmd

    try:
        res = run_bass_kernel_spmd(
            _get_nc(), in_maps, core_ids=list(range(NCORES))
        )
    except Exception:
        # transient NRT_EXEC_UNIT_UNRECOVERABLE from a previously wedged
        # device clears on redispatch
        time.sleep(5)
        res = run_bass_kernel_spmd(
            _get_nc(), in_maps, core_ids=list(range(NCORES))
        )
    _CACHE["last_res"] = res
    outs = res.results

    out = np.zeros((B_, H_, S_, D_), np.float32)
    for cid in range(NCORES):
        b = cid // 4
        hs = [2 * (cid % 4), 2 * (cid % 4) + 1]
        for p, h in enumerate(hs):
            ot = np.asarray(outs[cid]["ot"][p], np.float32)  # [65, 512]
            out[b, h] = (ot[:64] / ot[64:65]).T
    return out


# revision 29
# speedup vs baseline: 1.0689x; 1.0056x over previous
"""Sparse (class-gated bilinear) attention kernel for TRN2, 8 NeuronCores.

Problem shapes (hardcoded): b=2, h=8, s=512, d=64, C=8 classes, B=4 bases.

Math (per b,h):
  W1e[c] = (sum_B softmax(alpha1)[c,B,h] * W1[B,h]) / sqrt(d)   (host)
  W2e[c] = sum_B softmax(alpha2)[c,B,h] * W2[B,h]               (host)
  UT_c   = W1e[c]^T-contraction:  UT_c[n,i] = sum_m W1e[c][m,n] * Q[i,m]
  ST_c   = ST_c[j,i] = sum_n K[j,n] * UT_c[n,i]                 (PE, fp32r)
  scoresT[j,i] = ST_{bmat[i,j]}[j,i] + rpb[i,j]                 (DVE select)
  E = exp(scoresT)           (no max-subtraction needed; |scores| < ~40)
  t_c[j,D] = sum_d V[j,d] W2e[c][d,D]                           (PE)
  outT[D,i] = sum_c sum_j t_c[j,D] * (E . mask_c)[j,i]          (PE, bf16)
  Z[i] = sum_j E[j,i]                                           (PE ones-row)
  out[i,D] = outT[D,i] / Z[i]                                   (host)

Sharding: 16 (b,h) pairs over 8 cores; core k handles b=k//4,
heads (2*(k%4), 2*(k%4)+1). b_mat shared by both heads of a core.
"""

import os
import sys

import numpy as np

if "/opt/trn_rl_repo" not in sys.path:
    sys.path.insert(0, "/opt/trn_rl_repo")

import ml_dtypes

B_, H_, S_, D_, C_ = 2, 8, 512, 64, 8
NCORES = 8
JT = S_ // 128  # 4 j-tiles

# Selection chain dtype is fp32 (reads ST PSUM directly); split/output
# matmul side runs in ELEM (bf16 — contributes only ~0.2% rel err).
ELEM = "bfloat16"

_CACHE = {}


def _softmax(a, axis):
    e = np.exp(a - a.max(axis=axis, keepdims=True))
    return e / e.sum(axis=axis, keepdims=True)


def _build_nc():
    import concourse.bass as bass  # noqa: F401
    import concourse.mybir as mybir
    from concourse import bacc
    from concourse.tile import TileContext

    f32 = mybir.dt.float32
    f32r = mybir.dt.float32r
    f16 = mybir.dt.float16
    ebt = mybir.dt.bfloat16 if ELEM == "bfloat16" else mybir.dt.float32

    nc = bacc.Bacc("TRN2", target_bir_lowering=False, debug=False)

    qt_d = nc.dram_tensor("qt", [2, 64, 512], f32r, kind="ExternalInput").ap()
    kt_d = nc.dram_tensor("kt", [2, 64, 512], f32r, kind="ExternalInput").ap()
    vt_d = nc.dram_tensor("vt", [2, 64, 512], f32r, kind="ExternalInput").ap()
    w1_d = nc.dram_tensor("w1", [2, 64, 512], f32r, kind="ExternalInput").ap()
    w2_d = nc.dram_tensor("w2", [2, 64, 512], f32r, kind="ExternalInput").ap()
    erp_d = nc.dram_tensor("erp", [2, 512, 512], ebt, kind="ExternalInput").ap()
    bmt_d = nc.dram_tensor("bmt", [512, 512], ebt, kind="ExternalInput").ap()
    ot_d = nc.dram_tensor("ot", [2, 65, 512], f32, kind="ExternalOutput").ap()

    EXP = mybir.ActivationFunctionType.Exp
    EQ = mybir.AluOpType.is_equal

    with TileContext(nc) as tc:
        with (
            tc.tile_pool(name="const", bufs=1) as cpool,
            tc.tile_pool(name="inp", bufs=1) as ipool,
            tc.tile_pool(name="mask", bufs=1) as mpool,
            tc.tile_pool(name="work", bufs=5) as wpool,
            tc.tile_pool(name="ec", bufs=24) as epool,
            tc.tile_pool(name="pst", bufs=6, space="PSUM") as pst,
            tc.tile_pool(name="pacc", bufs=1, space="PSUM") as pacc,
        ):
            ones = cpool.tile([128, 1], ebt, tag="ones")
            nc.vector.memset(ones, 1.0)

            # per-class uint16 masks from b_mat^T, shared by both heads on
            # the core; used by copy_predicated (chain) AND split muls
            imasks = [[None] * C_ for _ in range(JT)]
            for jt in range(JT):
                bt = ipool.tile([128, 512], ebt, tag=f"bmt{jt}")
                nc.sync.dma_start(out=bt, in_=bmt_d[jt * 128 : (jt + 1) * 128, :])
                for c in range(C_):
                    im = mpool.tile([128, 512], mybir.dt.uint16, tag=f"i{jt}_{c}")
                    nc.vector.tensor_scalar(im, bt, float(c), None, EQ)
                    imasks[jt][c] = im

            qt, kt, vt, w1, w2 = {}, {}, {}, {}, {}
            ut, tsb = {}, {}
            ot_ps, z_ps = {}, {}
            for p in range(2):
                qt[p] = ipool.tile([64, 512], f32r, tag=f"qt{p}", name=f"qt{p}")
                nc.sync.dma_start(out=qt[p], in_=qt_d[p])
                kt[p] = ipool.tile([64, 512], f32r, tag=f"kt{p}", name=f"kt{p}")
                nc.sync.dma_start(out=kt[p], in_=kt_d[p])
                vt[p] = ipool.tile([64, 512], f32r, tag=f"vt{p}", name=f"vt{p}")
                nc.sync.dma_start(out=vt[p], in_=vt_d[p])
                w1[p] = ipool.tile([64, 512], f32r, tag=f"w1{p}", name=f"w1{p}")
                nc.sync.dma_start(out=w1[p], in_=w1_d[p])
                w2[p] = ipool.tile([64, 512], f32r, tag=f"w2{p}", name=f"w2{p}")
                nc.sync.dma_start(out=w2[p], in_=w2_d[p])

                # UT_c = W1e[c].T-contract @ Q^T : [64, 512] each
                ut[p] = []
                for c in range(C_):
                    up = pst.tile([128, 512], mybir.dt.float32, tag="st")
                    nc.tensor.matmul(
                        up[:64], w1[p][:, c * 64 : (c + 1) * 64], qt[p],
                        start=True, stop=True,
                    )
                    us = ipool.tile([64, 512], f32r, tag=f"ut{p}_{c}")
                    nc.any.tensor_copy(out=us, in_=up[:64])
                    ut[p].append(us)

                # t_all[j-tile] = V-tile @ W2cat : [128, (c,D)=512]
                tsb[p] = []
                for jt in range(JT):
                    tp = pst.tile([128, 512], mybir.dt.float32, tag="st")
                    nc.tensor.matmul(
                        tp, vt[p][:, jt * 128 : (jt + 1) * 128], w2[p],
                        start=True, stop=True,
                    )
                    ts = ipool.tile([128, 512], ebt, tag=f"t{p}_{jt}")
                    nc.any.tensor_copy(out=ts, in_=tp)
                    tsb[p].append(ts)

                ot_ps[p] = pacc.tile([65, 512], mybir.dt.float32, tag=f"o{p}", name=f"ot{p}")
                z_ps[p] = ot_ps[p][64:65]

            # Interleaved (jt, p) steps. Output/Z matmuls for step s are
            # emitted during step s+1 so they never block the next step's
            # ST matmuls in the in-order PE stream.
            pending = None

            def flush_pending():
                et_, ecs_, p_, jt_ = pending
                nc.tensor.matmul(
                    z_ps[p_], ones, et_,
                    start=(jt_ == 0), stop=(jt_ == JT - 1),
                    skip_group_check=True,
                )
                for c in range(C_):
                    nc.tensor.matmul(
                        ot_ps[p_][:64], tsb[p_][jt_][:, c * 64 : (c + 1) * 64],
                        ecs_[c],
                        start=(jt_ == 0 and c == 0),
                        stop=(jt_ == JT - 1 and c == C_ - 1),
                        skip_group_check=True,
                    )

            for jt in range(JT):
                for p in range(2):
                    rp = wpool.tile([128, 512], ebt, tag="rpb")
                    nc.sync.dma_start(
                        out=rp, in_=erp_d[p, jt * 128 : (jt + 1) * 128, :]
                    )
                    # ST matmuls -> PSUM; fp32 selection chain reads the
                    # PSUM banks directly. ACT does the class-0 seed copy.
                    sc = wpool.tile([128, 512], f32, tag="sc")
                    for c in range(C_):
                        sp = pst.tile([128, 512], mybir.dt.float32, tag="st")
                        nc.tensor.matmul(
                            sp, kt[p][:, jt * 128 : (jt + 1) * 128], ut[p][c],
                            start=True, stop=True,
                        )
                        if c == 0:
                            nc.scalar.copy(sc, sp)
                        else:
                            nc.vector.copy_predicated(sc, imasks[jt][c], sp)

                    eraw = wpool.tile([128, 512], ebt, tag="eraw")
                    nc.scalar.activation(eraw, sc, EXP)
                    et = wpool.tile([128, 512], ebt, tag="et")
                    nc.vector.tensor_mul(et, eraw, rp)

                    ecs = []
                    for c in range(C_):
                        ec = epool.tile([128, 512], ebt, tag="ec")
                        eng = nc.gpsimd if c in (1, 3, 5, 7) else nc.vector
                        eng.tensor_mul(ec, et, imasks[jt][c])
                        ecs.append(ec)

                    if pending is not None:
                        flush_pending()
                    pending = (et, ecs, p, jt)
            flush_pending()

            for p in range(2):
                os_ = wpool.tile([65, 512], mybir.dt.float32, tag="os")
                nc.scalar.copy(os_, ot_ps[p])
                nc.sync.dma_start(out=ot_d[p], in_=os_)

    nc.compile()
    return nc


def _get_nc():
    if "nc" not in _CACHE:
        _CACHE["nc"] = _build_nc()
    return _CACHE["nc"]


def kernel(**inputs):
    q = np.asarray(inputs["query"], np.float32)
    k = np.asarray(inputs["key"], np.float32)
    v = np.asarray(inputs["value"], np.float32)
    bm = np.asarray(inputs["b_mat"])
    rpb = np.asarray(inputs["rpb"], np.float32)
    W1 = np.asarray(inputs["W1"], np.float32)
    a1 = np.asarray(inputs["alpha1"], np.float32)
    W2 = np.asarray(inputs["W2"], np.float32)
    a2 = np.asarray(inputs["alpha2"], np.float32)
    mask = np.asarray(inputs["mask"])

    W1e = np.einsum("Bhmn,CBh->Chmn", W1, _softmax(a1, 1)) / np.sqrt(D_)
    W2e = np.einsum("BhdD,CBh->ChdD", W2, _softmax(a2, 1))

    eb = ml_dtypes.bfloat16 if ELEM == "bfloat16" else np.float32
    # additive -inf pair mask would go here; spec guarantees mask == ones
    assert mask.all(), "kernel assumes all-ones mask (spec fill=ones)"

    in_maps = []
    for cid in range(NCORES):
        b = cid // 4
        hs = [2 * (cid % 4), 2 * (cid % 4) + 1]
        qt = np.stack([q[b, h].T for h in hs]).astype(np.float32)
        kt = np.stack([k[b, h].T for h in hs]).astype(np.float32)
        vt = np.stack([v[b, h].T for h in hs]).astype(np.float32)
        # [m, C, n] -> [64, 512] per head
        w1 = np.stack(
            [W1e[:, h].transpose(1, 0, 2).reshape(64, 512) for h in hs]
        ).astype(np.float32)
        w2 = np.stack(
            [W2e[:, h].transpose(1, 0, 2).reshape(64, 512) for h in hs]
        ).astype(np.float32)
        erp = np.exp(np.stack([rpb[b, h].T for h in hs])).astype(
            ml_dtypes.bfloat16
        )
        bmt = bm[b].T.astype(np.float32).astype(eb)
        in_maps.append(
            {"qt": qt, "kt": kt, "vt": vt, "w1": w1, "w2": w2,
             "erp": erp, "bmt": bmt}
        )

    import time

    from concourse.bass_utils import run_bass_kernel_spmd

    try:
        res = run_bass_kernel_spmd(
            _get_nc(), in_maps, core_ids=list(range(NCORES))
        )
    except Exception:
        # transient NRT_EXEC_UNIT_UNRECOVERABLE from a previously wedged
        # device clears on redispatch
        time.sleep(5)
        res = run_bass_kernel_spmd(
            _get_nc(), in_maps, core_ids=list(range(NCORES))
        )
    _CACHE["last_res"] = res
    outs = res.results

    out = np.zeros((B_, H_, S_, D_), np.float32)
    for cid in range(NCORES):
        b = cid // 4
        hs = [2 * (cid % 4), 2 * (cid % 4) + 1]
        for p, h in enumerate(hs):
            ot = np.asarray(outs[cid]["ot"][p], np.float32)  # [65, 512]
            out[b, h] = (ot[:64] / ot[64:65]).T
    return out


# revision 30
# speedup vs baseline: 1.0777x; 1.0082x over previous
"""Sparse (class-gated bilinear) attention kernel for TRN2, 8 NeuronCores.

Problem shapes (hardcoded): b=2, h=8, s=512, d=64, C=8 classes, B=4 bases.

Math (per b,h):
  W1e[c] = (sum_B softmax(alpha1)[c,B,h] * W1[B,h]) / sqrt(d)   (host)
  W2e[c] = sum_B softmax(alpha2)[c,B,h] * W2[B,h]               (host)
  UT_c   = W1e[c]^T-contraction:  UT_c[n,i] = sum_m W1e[c][m,n] * Q[i,m]
  ST_c   = ST_c[j,i] = sum_n K[j,n] * UT_c[n,i]                 (PE, fp32r)
  scoresT[j,i] = ST_{bmat[i,j]}[j,i] + rpb[i,j]                 (DVE select)
  E = exp(scoresT)           (no max-subtraction needed; |scores| < ~40)
  t_c[j,D] = sum_d V[j,d] W2e[c][d,D]                           (PE)
  outT[D,i] = sum_c sum_j t_c[j,D] * (E . mask_c)[j,i]          (PE, bf16)
  Z[i] = sum_j E[j,i]                                           (PE ones-row)
  out[i,D] = outT[D,i] / Z[i]                                   (host)

Sharding: 16 (b,h) pairs over 8 cores; core k handles b=k//4,
heads (2*(k%4), 2*(k%4)+1). b_mat shared by both heads of a core.
"""

import os
import sys

import numpy as np

if "/opt/trn_rl_repo" not in sys.path:
    sys.path.insert(0, "/opt/trn_rl_repo")

import ml_dtypes

B_, H_, S_, D_, C_ = 2, 8, 512, 64, 8
NCORES = 8
JT = S_ // 128  # 4 j-tiles

# Selection chain dtype is fp32 (reads ST PSUM directly); split/output
# matmul side runs in ELEM (bf16 — contributes only ~0.2% rel err).
ELEM = "bfloat16"

_CACHE = {}


def _softmax(a, axis):
    e = np.exp(a - a.max(axis=axis, keepdims=True))
    return e / e.sum(axis=axis, keepdims=True)


def _build_nc():
    import concourse.bass as bass  # noqa: F401
    import concourse.mybir as mybir
    from concourse import bacc
    from concourse.tile import TileContext

    f32 = mybir.dt.float32
    f32r = mybir.dt.float32r
    f16 = mybir.dt.float16
    ebt = mybir.dt.bfloat16 if ELEM == "bfloat16" else mybir.dt.float32

    nc = bacc.Bacc("TRN2", target_bir_lowering=False, debug=False)

    qt_d = nc.dram_tensor("qt", [2, 64, 512], f32r, kind="ExternalInput").ap()
    kt_d = nc.dram_tensor("kt", [2, 64, 512], f32r, kind="ExternalInput").ap()
    vt_d = nc.dram_tensor("vt", [2, 64, 512], f32r, kind="ExternalInput").ap()
    w1_d = nc.dram_tensor("w1", [2, 64, 512], f32r, kind="ExternalInput").ap()
    w2_d = nc.dram_tensor("w2", [2, 64, 512], f32r, kind="ExternalInput").ap()
    erp_d = nc.dram_tensor("erp", [2, 512, 512], ebt, kind="ExternalInput").ap()
    bmt_d = nc.dram_tensor("bmt", [512, 512], ebt, kind="ExternalInput").ap()
    ot_d = nc.dram_tensor("ot", [2, 65, 512], f32, kind="ExternalOutput").ap()

    EXP = mybir.ActivationFunctionType.Exp
    EQ = mybir.AluOpType.is_equal

    with TileContext(nc) as tc:
        with (
            tc.tile_pool(name="const", bufs=1) as cpool,
            tc.tile_pool(name="inp", bufs=1) as ipool,
            tc.tile_pool(name="mask", bufs=1) as mpool,
            tc.tile_pool(name="work", bufs=5) as wpool,
            tc.tile_pool(name="ec", bufs=24) as epool,
            tc.tile_pool(name="pst", bufs=6, space="PSUM") as pst,
            tc.tile_pool(name="pacc", bufs=1, space="PSUM") as pacc,
        ):
            ones = cpool.tile([128, 1], ebt, tag="ones")
            nc.vector.memset(ones, 1.0)

            # per-class uint16 masks from b_mat^T, shared by both heads on
            # the core; used by copy_predicated (chain) AND split muls
            imasks = [[None] * C_ for _ in range(JT)]
            mpairs = [[None] * (C_ // 2) for _ in range(JT)]
            for jt in range(JT):
                bt = ipool.tile([128, 512], ebt, tag=f"bmt{jt}")
                nc.sync.dma_start(out=bt, in_=bmt_d[jt * 128 : (jt + 1) * 128, :])
                for q in range(C_ // 2):
                    mp = mpool.tile(
                        [128, 1024], mybir.dt.uint16, tag=f"i{jt}_{q}",
                        name=f"i{jt}_{q}",
                    )
                    for h_ in range(2):
                        c = 2 * q + h_
                        im = mp[:, h_ * 512 : (h_ + 1) * 512]
                        nc.vector.tensor_scalar(im, bt, float(c), None, EQ)
                        imasks[jt][c] = im
                    mpairs[jt][q] = mp

            qt, kt, vt, w1, w2 = {}, {}, {}, {}, {}
            ut, tsb = {}, {}
            ot_ps, z_ps = {}, {}
            for p in range(2):
                qt[p] = ipool.tile([64, 512], f32r, tag=f"qt{p}", name=f"qt{p}")
                nc.sync.dma_start(out=qt[p], in_=qt_d[p])
                kt[p] = ipool.tile([64, 512], f32r, tag=f"kt{p}", name=f"kt{p}")
                nc.sync.dma_start(out=kt[p], in_=kt_d[p])
                vt[p] = ipool.tile([64, 512], f32r, tag=f"vt{p}", name=f"vt{p}")
                nc.sync.dma_start(out=vt[p], in_=vt_d[p])
                w1[p] = ipool.tile([64, 512], f32r, tag=f"w1{p}", name=f"w1{p}")
                nc.sync.dma_start(out=w1[p], in_=w1_d[p])
                w2[p] = ipool.tile([64, 512], f32r, tag=f"w2{p}", name=f"w2{p}")
                nc.sync.dma_start(out=w2[p], in_=w2_d[p])

                # UT_c = W1e[c].T-contract @ Q^T : [64, 512] each
                ut[p] = []
                for c in range(C_):
                    up = pst.tile([128, 512], mybir.dt.float32, tag="st")
                    nc.tensor.matmul(
                        up[:64], w1[p][:, c * 64 : (c + 1) * 64], qt[p],
                        start=True, stop=True,
                    )
                    us = ipool.tile([64, 512], f32r, tag=f"ut{p}_{c}")
                    nc.any.tensor_copy(out=us, in_=up[:64])
                    ut[p].append(us)

                # t_all[j-tile] = V-tile @ W2cat : [128, (c,D)=512]
                tsb[p] = []
                for jt in range(JT):
                    tp = pst.tile([128, 512], mybir.dt.float32, tag="st")
                    nc.tensor.matmul(
                        tp, vt[p][:, jt * 128 : (jt + 1) * 128], w2[p],
                        start=True, stop=True,
                    )
                    ts = ipool.tile([128, 512], ebt, tag=f"t{p}_{jt}")
                    nc.any.tensor_copy(out=ts, in_=tp)
                    tsb[p].append(ts)

                ot_ps[p] = pacc.tile([65, 512], mybir.dt.float32, tag=f"o{p}", name=f"ot{p}")
                z_ps[p] = ot_ps[p][64:65]

            # Interleaved (jt, p) steps. Output/Z matmuls for step s are
            # emitted during step s+1 so they never block the next step's
            # ST matmuls in the in-order PE stream.
            pending = None

            def flush_pending():
                et_, ecs_, p_, jt_ = pending
                nc.tensor.matmul(
                    z_ps[p_], ones, et_,
                    start=(jt_ == 0), stop=(jt_ == JT - 1),
                    skip_group_check=True,
                )
                for c in range(C_):
                    nc.tensor.matmul(
                        ot_ps[p_][:64], tsb[p_][jt_][:, c * 64 : (c + 1) * 64],
                        ecs_[c // 2][:, (c % 2) * 512 : (c % 2 + 1) * 512],
                        start=(jt_ == 0 and c == 0),
                        stop=(jt_ == JT - 1 and c == C_ - 1),
                        skip_group_check=True,
                    )

            for jt in range(JT):
                for p in range(2):
                    rp = wpool.tile([128, 512], ebt, tag="rpb")
                    nc.sync.dma_start(
                        out=rp, in_=erp_d[p, jt * 128 : (jt + 1) * 128, :]
                    )
                    # ST matmuls -> PSUM; fp32 selection chain reads the
                    # PSUM banks directly. ACT does the class-0 seed copy.
                    sc = wpool.tile([128, 512], f32, tag="sc")
                    for c in range(C_):
                        sp = pst.tile([128, 512], mybir.dt.float32, tag="st")
                        nc.tensor.matmul(
                            sp, kt[p][:, jt * 128 : (jt + 1) * 128], ut[p][c],
                            start=True, stop=True,
                        )
                        if c == 0:
                            nc.scalar.copy(sc, sp)
                        else:
                            nc.vector.copy_predicated(sc, imasks[jt][c], sp)

                    eraw = wpool.tile([128, 512], ebt, tag="eraw")
                    nc.scalar.activation(eraw, sc, EXP)
                    et = wpool.tile([128, 512], ebt, tag="et")
                    nc.vector.tensor_mul(et, eraw, rp)

                    etb = et[:, None, :].to_broadcast([128, 2, 512])
                    ecs = []
                    for q in range(C_ // 2):
                        ec2 = epool.tile(
                            [128, 1024], ebt, tag="ec", name=f"ec{q}"
                        )
                        eng = nc.gpsimd if q in (1, 3) else nc.vector
                        eng.tensor_mul(
                            ec2.rearrange("p (two f) -> p two f", two=2),
                            etb,
                            mpairs[jt][q].rearrange(
                                "p (two f) -> p two f", two=2
                            ),
                        )
                        ecs.append(ec2)

                    if pending is not None:
                        flush_pending()
                    pending = (et, ecs, p, jt)
            flush_pending()

            for p in range(2):
                os_ = wpool.tile([65, 512], mybir.dt.float32, tag="os")
                nc.scalar.copy(os_, ot_ps[p])
                nc.sync.dma_start(out=ot_d[p], in_=os_)

    nc.compile()
    return nc


def _get_nc():
    if "nc" not in _CACHE:
        _CACHE["nc"] = _build_nc()
    return _CACHE["nc"]


def kernel(**inputs):
    q = np.asarray(inputs["query"], np.float32)
    k = np.asarray(inputs["key"], np.float32)
    v = np.asarray(inputs["value"], np.float32)
    bm = np.asarray(inputs["b_mat"])
    rpb = np.asarray(inputs["rpb"], np.float32)
    W1 = np.asarray(inputs["W1"], np.float32)
    a1 = np.asarray(inputs["alpha1"], np.float32)
    W2 = np.asarray(inputs["W2"], np.float32)
    a2 = np.asarray(inputs["alpha2"], np.float32)
    mask = np.asarray(inputs["mask"])

    W1e = np.einsum("Bhmn,CBh->Chmn", W1, _softmax(a1, 1)) / np.sqrt(D_)
    W2e = np.einsum("BhdD,CBh->ChdD", W2, _softmax(a2, 1))

    eb = ml_dtypes.bfloat16 if ELEM == "bfloat16" else np.float32
    # additive -inf pair mask would go here; spec guarantees mask == ones
    assert mask.all(), "kernel assumes all-ones mask (spec fill=ones)"

    in_maps = []
    for cid in range(NCORES):
        b = cid // 4
        hs = [2 * (cid % 4), 2 * (cid % 4) + 1]
        qt = np.stack([q[b, h].T for h in hs]).astype(np.float32)
        kt = np.stack([k[b, h].T for h in hs]).astype(np.float32)
        vt = np.stack([v[b, h].T for h in hs]).astype(np.float32)
        # [m, C, n] -> [64, 512] per head
        w1 = np.stack(
            [W1e[:, h].transpose(1, 0, 2).reshape(64, 512) for h in hs]
        ).astype(np.float32)
        w2 = np.stack(
            [W2e[:, h].transpose(1, 0, 2).reshape(64, 512) for h in hs]
        ).astype(np.float32)
        erp = np.exp(np.stack([rpb[b, h].T for h in hs])).astype(
            ml_dtypes.bfloat16
        )
        bmt = bm[b].T.astype(np.float32).astype(eb)
        in_maps.append(
            {"qt": qt, "kt": kt, "vt": vt, "w1": w1, "w2": w2,
             "erp": erp, "bmt": bmt}
        )

    import time

    from concourse.bass_utils import run_bass_kernel_spmd

    try:
        res = run_bass_kernel_spmd(
            _get_nc(), in_maps, core_ids=list(range(NCORES))
        )
    except Exception:
        # transient NRT_EXEC_UNIT_UNRECOVERABLE from a previously wedged
        # device clears on redispatch
        time.sleep(5)
        res = run_bass_kernel_spmd(
            _get_nc(), in_maps, core_ids=list(range(NCORES))
        )
    _CACHE["last_res"] = res
    outs = res.results

    out = np.zeros((B_, H_, S_, D_), np.float32)
    for cid in range(NCORES):
        b = cid // 4
        hs = [2 * (cid % 4), 2 * (cid % 4) + 1]
        for p, h in enumerate(hs):
            ot = np.asarray(outs[cid]["ot"][p], np.float32)  # [65, 512]
            out[b, h] = (ot[:64] / ot[64:65]).T
    return out
